# revision 1
# baseline (speedup 1.0000x reference)
"""Trainium2 Bass kernel for nn_EquivariantConvolutionBlock (sparse 5^3 equivariant
conv + gate + batchnorm over 300k voxels in a 128^3 grid), SPMD over 8 NeuronCores.

v2: fp8 gather path.
- Host folds the e3nn tensor-product kernel into per-window stationary matrices;
  the center cell + e3nn Linear self-connection are split out into a separate
  fp16 matmul (they dominate magnitude, so they stay high precision) fed by a
  dense per-plane feature tile (regular DMA, no gather).
- Neighborhood B-volume is fp8 (features x16): each 512B row = a [4dy x 4dz]
  block of 32-ch features; one dma_gather element feeds 2 DoubleRow matmuls
  (fp8 perf mode, 2 k-subtiles each) per window-block.
- Weight quantization error is cancelled by a second fp8 stationary bank
  holding the rounding residual (accumulates into the same PSUM group).
- Gathers are merged per same-dx window-block group and chopped at 768
  indices (the SWDGE ring cap) to amortize the ~1us fixed cost per gather;
  a SINGLE SWDGE queue is used - multiple queues nondeterministically
  corrupt concurrent gathers on this HW. Gated pre-BN activations stay
  SBUF-resident (no DRAM spill); BatchNorm batch stats AllReduce across
  cores; normalization applied from SBUF.
"""
import sys

sys.path.insert(0, "/opt/trn_rl_repo")

import os
import numpy as np
from contextlib import ExitStack

import concourse.bass as bass
import concourse.bacc as bacc
import concourse.tile as tile
import concourse.mybir as mybir
from concourse.bass_utils import run_bass_kernel_spmd

F8 = mybir.dt.float8e4
F16 = mybir.dt.float16
F32 = mybir.dt.float32
I16 = mybir.dt.int16
NP8 = mybir.dt.np(F8)

N = 300000
GRID = 128
NCORES = 8
EPS = 1e-5
PPC = 16          # planes per core
NOCC = os.environ.get("NOCC", "0") == "1"
YQ = 132          # y-block-start axis
SD = 132          # z-window-start axis
PLANE_ROWS = SD * YQ          # 17424
BROWS = 20 * PLANE_ROWS       # B-volume rows per core (20 x-planes incl halo)
TCOLS = 512
PAD_IDX = 130     # (s=0, yq=130): all-zero elem
FSCALE = 16.0     # feature scale into fp8

# window-blocks: (dx, ady, adz); block covers dy in [ady, ady+3], dz in [adz, adz+3]
WBS = [(-2, -1, -1),
       (-1, -2, -2), (-1, -1, -1),
       (0, -2, -2), (0, -1, -1),
       (1, -2, -2), (1, -1, -1),
       (2, -1, -1)]

_COMPILED = None


# ---------------------------------------------------------------- host math

def _soft_unit_step(t):
    out = np.zeros_like(t)
    m = t > 0
    out[m] = np.exp(-1.0 / t[m])
    return out


def _make_ker_by_off(tp_weight):
    ax = np.arange(-2, 3.0)
    lat = np.stack(np.meshgrid(ax, ax, ax, indexing="ij"), -1).reshape(-1, 3)
    d = np.linalg.norm(lat, axis=-1)
    values = np.linspace(0.0, 2.5, 5)[1:-1]
    step = 2.5 / 4
    diff = (d[..., None] - values) / step
    emb = 1.14136 * float(np.e ** 2) * _soft_unit_step(diff + 1.0) * _soft_unit_step(1.0 - diff)
    w = (emb @ tp_weight.astype(np.float64)) / 125.0
    w1, w2, w3, w4, w5, w6 = [w[:, i * 128:(i + 1) * 128].reshape(-1, 8, 16) for i in range(6)]
    unit = np.where(d[:, None] > 0, lat / np.where(d > 0, d, 1.0)[:, None], 0.0)
    y1 = np.sqrt(3.0) * unit
    A = 0.25
    B = A / np.sqrt(3.0)
    Cc = 0.25
    M_ss = A * w1
    M_vs = (B * np.einsum("xi,xuw->xuiw", y1, w2)).reshape(-1, 24, 16)
    M_sg = A * w3
    M_vg = (B * np.einsum("xi,xuw->xuiw", y1, w4)).reshape(-1, 24, 16)
    M_sv = (Cc * np.einsum("xk,xuw->xuwk", y1, w5)).reshape(-1, 8, 48)
    M_vv = (Cc * np.einsum("xuw,ik->xuiwk", w6, np.eye(3))).reshape(-1, 24, 48)
    top = np.concatenate([M_ss, M_sg, M_sv], -1)
    bot = np.concatenate([M_vs, M_vg, M_vv], -1)
    ker = np.concatenate([top, bot], 1)            # [125,32,80]
    return {tuple(int(v) for v in lat[i]): ker[i] for i in range(125)}


def _center_matrix(kbo, Ws1, Ws2, Wv):
    """Center conv cell + e3nn Linear self-connection, [32,80]."""
    inv = 1.0 / np.sqrt(8.0)
    Wsc = np.zeros((32, 80))
    Wsc[0:8, 0:16] = Ws1 * inv
    Wsc[0:8, 16:32] = Ws2 * inv
    u, w_ = np.meshgrid(np.arange(8), np.arange(16), indexing="ij")
    for i in range(3):
        Wsc[8 + u * 3 + i, 32 + w_ * 3 + i] = Wv * inv
    return kbo[(0, 0, 0)] + Wsc


def _active(dx, dy, dz):
    d2 = dx * dx + dy * dy + dz * dz
    return 0 < d2 <= 6


def _build_stationaries(kbo):
    """[128, 8*4*112] f64: slot (wbi*4+q) = column (dx, ady+q), rows 32k+c = dz=adz+k.
    Center cell excluded. Output channel layout: 0:16 s | 32:48 gates | 64:112 v."""
    kers = np.zeros((128, len(WBS) * 4 * 112))
    for wbi, (dx, ady, adz) in enumerate(WBS):
        for q in range(4):
            dy = ady + q
            c0 = (wbi * 4 + q) * 112
            for k in range(4):
                dz = adz + k
                if abs(dy) > 2 or abs(dz) > 2 or not _active(dx, dy, dz):
                    continue
                # first covering block owns the cell
                own = None
                for i, (wdx, a, b) in enumerate(WBS):
                    if wdx == dx and a <= dy <= a + 3 and b <= dz <= b + 3:
                        own = i
                        break
                if own != wbi:
                    continue
                m = kbo[(dx, dy, dz)]
                kers[32 * k:32 * (k + 1), c0 + 0:c0 + 16] = m[:, 0:16]
                kers[32 * k:32 * (k + 1), c0 + 32:c0 + 48] = m[:, 16:32]
                kers[32 * k:32 * (k + 1), c0 + 64:c0 + 112] = m[:, 32:80]
    return kers


def _pack_doublerow(kers):
    """kers [128, 8*4*112] -> [128, 8*2*2*112] fp8-ready DoubleRow layout.
    st[p, wb, q', i, m] = kers[(2p+i)%128, slot(wb, 2q' + (2p+i)//128), m]."""
    k4 = kers.reshape(128, len(WBS), 4, 112)
    st = np.zeros((128, len(WBS), 2, 2, 112), kers.dtype)
    p = np.arange(128)
    for i in range(2):
        kk = 2 * p + i
        row, hi = kk % 128, kk // 128
        for qp in range(2):
            # q = 2*qp + hi (per-partition)
            st[p, :, qp, i, :] = k4[row, :, 2 * qp + hi, :]
    return st.reshape(128, -1)


def _wrap_idx(flat):
    w16 = flat.reshape(-1, 16).T.astype(np.int16)
    return np.tile(w16, (8, 1))


# ---------------------------------------------------------------- device program

def _build_program(TPP):
    PCOLS = TPP * TCOLS
    NTILE = PPC * TPP
    NPC = PPC * PCOLS
    IDXW = PPC * len(WBS) * PCOLS // 16
    # SWDGE ring caps a single dma_gather at <=768 indices (1024 fails on HW)
    GMAX = int(os.environ.get("KGMAX", "2560"))

    def chop(total):
        out, off = [], 0
        while off < total:
            hn = min(GMAX, total - off)
            out.append((off, hn))
            off += hn
        return out

    # same-dx window-blocks share a gather base region; their idx streams are
    # adjacent, so one gather can span both (fewer SWDGE fixed costs)
    GROUPS = [(-2, [0]), (-1, [1, 2]), (0, [3, 4]), (1, [5, 6]), (2, [7])]
    # The final group carries each psum tile's stop flag, so chop it at tile
    # granularity: tiles then unlock post-processing incrementally instead of
    # waiting for one whole-plane gather to land.
    gchunks = [chop(len(w) * PCOLS) for _, w in GROUPS]
    last_chunk = [max(ci for ci, (o, n) in enumerate(gchunks[-1])
                      if o < (t + 1) * TCOLS and o + n > t * TCOLS)
                  for t in range(TPP)]

    # CRITICAL: >1 SWDGE queue nondeterministically corrupts concurrent
    # gathers on this HW (only one queue context is actually allocated);
    # a single queue is bit-exact across runs.
    NQ = int(os.environ.get("KQ", "1"))
    nc = bacc.Bacc("TRN2", target_bir_lowering=False, debug=False,
                   num_devices=NCORES, num_swdge_queues=NQ)
    B_t = nc.dram_tensor("bvol", [BROWS, 512], F8, kind="ExternalInput").ap()
    IDX_t = nc.dram_tensor("idx", [128, IDXW], I16, kind="ExternalInput").ap()
    KER_t = nc.dram_tensor("kers", [128, 2 * len(WBS) * 2 * 2 * 112], F8,
                           kind="ExternalInput").ap()
    CK_t = nc.dram_tensor("cker", [32, 112], F16, kind="ExternalInput").ap()
    CF_t = nc.dram_tensor("cfeat", [32, NPC], F16, kind="ExternalInput").ap()
    E_t = nc.dram_tensor("emat", [16, 48], F32, kind="ExternalInput").ap()
    BN_t = nc.dram_tensor("bn", [1, 48], F32, kind="ExternalInput").ap()
    DSC_t = nc.dram_tensor("dsc", [112, 1], F32, kind="ExternalInput").ap()
    OUT_t = nc.dram_tensor("out", [64, NPC], F32, kind="ExternalOutput").ap()

    gather_ct = 0

    with tile.TileContext(nc) as tc, ExitStack() as ctx:
        cpool = ctx.enter_context(tc.tile_pool(name="const", bufs=1))
        ipool = ctx.enter_context(tc.tile_pool(name="idxp", bufs=2))
        fpool = ctx.enter_context(tc.tile_pool(name="cfp", bufs=2))
        r0pool = ctx.enter_context(tc.tile_pool(name="rhs0", bufs=int(os.environ.get("KRB", "5"))))
        r1pool = ctx.enter_context(tc.tile_pool(name="rhs1", bufs=2))
        xpool = ctx.enter_context(tc.tile_pool(name="xs", bufs=2))
        spool = ctx.enter_context(tc.tile_pool(name="small", bufs=2))
        qpool = ctx.enter_context(tc.tile_pool(name="sq", bufs=2))
        vpool = ctx.enter_context(tc.tile_pool(name="sv", bufs=1))
        opool = ctx.enter_context(tc.tile_pool(name="outp", bufs=int(os.environ.get("KOB", "3"))))
        stpool = ctx.enter_context(tc.tile_pool(name="stats", bufs=1))
        pp = ctx.enter_context(tc.tile_pool(name="psA", bufs=6, space="PSUM"))
        pg = ctx.enter_context(tc.tile_pool(name="psB", bufs=2, space="PSUM"))
        dpool = ctx.enter_context(tc.tile_pool(name="dram", bufs=1, space="DRAM"))

        # two fp8 stationary banks: rounded weights + quantization residual
        kers = cpool.tile([128, 2 * len(WBS) * 2 * 2 * 112], F8)
        nc.sync.dma_start(kers[:], KER_t[:])
        cker = cpool.tile([32, 112], F16)
        nc.sync.dma_start(cker[:], CK_t[:])
        e48 = cpool.tile([48, 48], F32)
        nc.sync.dma_start(e48[32:48, :], E_t[:])
        bn = cpool.tile([1, 48], F32)
        nc.sync.dma_start(bn[:], BN_t[:])
        dsc = cpool.tile([112, 1], F32)
        nc.sync.dma_start(dsc[:], DSC_t[:])
        ones = cpool.tile([1, 1], F32)
        nc.vector.memset(ones[:], 1.0)

        stats_s = stpool.tile([16, NTILE], F32, tag="sts")
        stats_s2 = stpool.tile([16, NTILE], F32, tag="sts2")
        stats_v2 = stpool.tile([112, NTILE], F32, tag="stv2")

        # SBUF-resident gated pre-BN; rows 0:16 = s, rows 64:112 = v (engine
        # ops cannot shift partitions, so v stays on the conv output rows)
        svr = vpool.tile([112, NPC], F16, tag="svres")

        for j in range(PPC):
            ps = [pp.tile([112, TCOLS], F32, tag="convps", name=f"ps_{j}_{t}")
                  for t in range(TPP)]
            idxt = ipool.tile([128, len(WBS) * PCOLS // 16], I16, tag="idx",
                              name=f"ix_{j}")
            nc.sync.dma_start(idxt[:], IDX_t[:, j * len(WBS) * PCOLS // 16:
                                             (j + 1) * len(WBS) * PCOLS // 16])
            cft = fpool.tile([32, PCOLS], F16, tag="cf", name=f"cf_{j}")
            nc.sync.dma_start(cft[:], CF_t[:, j * PCOLS:(j + 1) * PCOLS])
            # center + self-connection matmuls open each accumulation group
            for t in range(TPP):
                nc.tensor.matmul(ps[t][:], cker[:],
                                 cft[:, t * TCOLS:(t + 1) * TCOLS],
                                 start=True, stop=False)
            for gi, (dx, gwbs) in enumerate(GROUPS):
                base = (j + 2 + dx) * PLANE_ROWS
                goff = gwbs[0] * PCOLS        # group start in the plane stream
                for ci, (coff, cn) in enumerate(gchunks[gi]):
                    rhs = (r0pool if cn == GMAX else r1pool).tile(
                        [128, 4, cn], F8, tag=f"rhs{cn}",
                        name=f"rh_{j}_{gi}_{ci}")
                    nc.gpsimd.dma_gather(
                        rhs[:], B_t[base: base + PLANE_ROWS, :],
                        idxt[:, (goff + coff) // 16: (goff + coff + cn) // 16],
                        cn, cn, 512, transpose=True,
                        queue_num=gather_ct % NQ,
                        single_packet=os.environ.get("KSP", "0") == "1",
                    )
                    gather_ct += 1
                    rq = rhs[:].rearrange("p a n -> p (a n)").rearrange(
                        "p (q n e) -> p q e n", q=2, e=2)
                    for wl, wbi in enumerate(gwbs):
                        # this wb's span within the group stream ∩ this chunk
                        s0c = max(wl * PCOLS, coff)
                        s1c = min((wl + 1) * PCOLS, coff + cn)
                        if s0c >= s1c:
                            continue
                        p0 = s0c - wl * PCOLS     # plane-column range
                        p1 = s1c - wl * PCOLS
                        for qp in range(2):
                            for bank in range(2):
                                k0 = (bank * len(WBS) * 2 + wbi * 2 + qp) * 2 * 112
                                stat = kers[:, k0: k0 + 2 * 112].rearrange(
                                    "p (i m) -> p i m", i=2)
                                for t in range(p0 // TCOLS,
                                               min(TPP, -(-p1 // TCOLS))):
                                    c0 = max(t * TCOLS, p0)
                                    c1 = min((t + 1) * TCOLS, p1)
                                    lo = c0 - p0 + s0c - coff
                                    nc.tensor.matmul(
                                        ps[t][:, c0 - t * TCOLS:
                                              c1 - t * TCOLS],
                                        stat,
                                        rq[:, qp, :, lo: lo + c1 - c0],
                                        start=False,
                                        stop=(gi == len(GROUPS) - 1
                                              and ci == last_chunk[t]
                                              and qp == 1 and bank == 1),
                                        perf_mode=mybir.MatmulPerfMode.DoubleRow,
                                    )
            # post-processing per 512-col tile
            for t in range(TPP):
                ti = j * TPP + t
                cols = slice(ti * TCOLS, (ti + 1) * TCOLS)
                xs = xpool.tile([112, TCOLS], F32, tag="xs")
                nc.scalar.activation(xs[:], ps[t][:],
                                     mybir.ActivationFunctionType.Copy,
                                     scale=dsc[:])
                sig = spool.tile([48, TCOLS], F32, tag="sig")
                nc.scalar.activation(sig[:], xs[0:48, :],
                                     mybir.ActivationFunctionType.Sigmoid)
                nc.vector.tensor_tensor(svr[0:16, cols], xs[0:16, :],
                                        sig[0:16, :], mybir.AluOpType.mult)
                gex = pg.tile([112, TCOLS], F32, tag="gexps")
                nc.tensor.matmul(gex[64:112, :], e48[32:48, :], sig[32:48, :],
                                 start=True, stop=True, tile_position=(32, 64))
                gexs = spool.tile([112, TCOLS], F32, tag="gexs")
                nc.scalar.activation(gexs[64:112, :], gex[64:112, :],
                                     mybir.ActivationFunctionType.Copy)
                nc.vector.tensor_tensor(svr[64:112, cols], xs[64:112, :],
                                        gexs[64:112, :], mybir.AluOpType.mult)
                nc.vector.tensor_reduce(stats_s[:, ti:ti + 1], svr[0:16, cols],
                                        mybir.AxisListType.X, mybir.AluOpType.add)
                sq1 = qpool.tile([16, TCOLS], F32, tag="sq1")
                nc.scalar.activation(sq1[:], svr[0:16, cols],
                                     mybir.ActivationFunctionType.Square,
                                     accum_out=stats_s2[:, ti:ti + 1])
                sq2 = qpool.tile([112, TCOLS], F32, tag="sq2")
                nc.scalar.activation(sq2[64:112, :], svr[64:112, cols],
                                     mybir.ActivationFunctionType.Square,
                                     accum_out=stats_v2[64:112, ti:ti + 1])

        # ---- batch statistics: reduce partials, AllReduce, finalize scales
        red_s = stpool.tile([16, 1], F32)
        nc.vector.tensor_reduce(red_s[:], stats_s[:], mybir.AxisListType.X,
                                mybir.AluOpType.add)
        red_s2 = stpool.tile([16, 1], F32)
        nc.vector.tensor_reduce(red_s2[:], stats_s2[:], mybir.AxisListType.X,
                                mybir.AluOpType.add)
        red_v2 = stpool.tile([112, 1], F32)
        nc.vector.tensor_reduce(red_v2[64:112, :], stats_v2[64:112, :],
                                mybir.AxisListType.X, mybir.AluOpType.add)
        cc_in = dpool.tile([1, 80], F32)
        cc_out = dpool.tile([1, 80], F32)
        nc.sync.dma_start(cc_in[0:1, 0:16], red_s[:])
        nc.sync.dma_start(cc_in[0:1, 16:32], red_s2[:])
        nc.sync.dma_start(cc_in[0:1, 32:80], red_v2[64:112, :])
        if not NOCC:
            nc.gpsimd.collective_compute(
                "AllReduce", mybir.AluOpType.add,
                replica_groups=[list(range(NCORES))],
                ins=[cc_in.opt()], outs=[cc_out.opt()],
            )
        st = stpool.tile([1, 80], F32)
        nc.sync.dma_start(st[:], cc_in[:] if NOCC else cc_out[:])

        # fused s-var | v-norm rsqrt: one [1,32] Newton chain instead of two
        mu = stpool.tile([1, 16], F32)
        nc.vector.tensor_scalar_mul(mu[:], st[0:1, 0:16], 1.0 / N)
        vv = stpool.tile([1, 32], F32)
        nc.vector.tensor_scalar_mul(vv[0:1, 0:16], st[0:1, 16:32], 1.0 / N)
        mumu = stpool.tile([1, 16], F32)
        nc.vector.tensor_tensor(mumu[:], mu[:], mu[:], mybir.AluOpType.mult)
        nc.vector.tensor_tensor(vv[0:1, 0:16], vv[0:1, 0:16], mumu[:],
                                mybir.AluOpType.subtract)
        v3 = st[0:1, 32:80].rearrange("p (g d) -> p g d", d=3)
        nc.vector.tensor_reduce(vv[0:1, 16:32], v3, mybir.AxisListType.X,
                                mybir.AluOpType.add)
        nc.vector.tensor_scalar_mul(vv[0:1, 16:32], vv[0:1, 16:32],
                                    1.0 / (3.0 * N))
        t32 = stpool.tile([1, 32], F32)
        nc.vector.tensor_scalar_add(t32[:], vv[:], EPS)
        r32 = stpool.tile([1, 32], F32)
        nc.vector.reciprocal(r32[:], t32[:])
        q32 = stpool.tile([1, 32], F32)
        nc.scalar.activation(q32[:], r32[:], mybir.ActivationFunctionType.Sqrt)
        qq = stpool.tile([1, 32], F32)
        nc.vector.tensor_tensor(qq[:], q32[:], q32[:], mybir.AluOpType.mult)
        nc.vector.tensor_tensor(qq[:], qq[:], t32[:], mybir.AluOpType.mult)
        nc.vector.tensor_scalar_mul(qq[:], qq[:], -0.5)
        nc.vector.tensor_scalar_add(qq[:], qq[:], 1.5)
        ab = stpool.tile([1, 32], F32)
        nc.vector.tensor_tensor(ab[:], q32[:], qq[:], mybir.AluOpType.mult)
        nc.vector.tensor_tensor(ab[:], ab[:], bn[0:1, 0:32],
                                mybir.AluOpType.mult)
        b_s = stpool.tile([1, 16], F32)
        nc.vector.tensor_tensor(b_s[:], mu[:], ab[0:1, 0:16],
                                mybir.AluOpType.mult)
        nc.vector.tensor_tensor(b_s[:], bn[0:1, 32:48], b_s[:],
                                mybir.AluOpType.subtract)
        a_vec = stpool.tile([1, 112], F32)
        nc.vector.memset(a_vec[:], 0.0)
        nc.vector.tensor_copy(a_vec[0:1, 0:16], ab[0:1, 0:16])
        av3 = a_vec[0:1, 64:112].rearrange("p (g d) -> p g d", d=3)
        avs = ab[0:1, 16:32].rearrange("p (g d) -> p g d", d=1)
        for i in range(3):
            nc.vector.tensor_copy(av3[:, :, i:i + 1], avs[:])
        b_vec = stpool.tile([1, 112], F32)
        nc.vector.memset(b_vec[:], 0.0)
        nc.vector.tensor_copy(b_vec[0:1, 0:16], b_s[:])
        abps = pg.tile([112, 1], F32, tag="gexps")
        nc.tensor.matmul(abps[:], a_vec[:], ones[:], start=True, stop=True)
        a_col = stpool.tile([112, 1], F32)
        nc.scalar.activation(a_col[:], abps[:], mybir.ActivationFunctionType.Copy)
        abps2 = pg.tile([112, 1], F32, tag="gexps")
        nc.tensor.matmul(abps2[:], b_vec[:], ones[:], start=True, stop=True)
        b_col = stpool.tile([112, 1], F32)
        nc.scalar.activation(b_col[:], abps2[:], mybir.ActivationFunctionType.Copy)

        # ---- apply normalization: out = sv * a + b (from SBUF)
        # alternate planes between ACT and DVE so the post-barrier tail halves
        for j in range(PPC):
            cols = slice(j * PCOLS, (j + 1) * PCOLS)
            outt = opool.tile([112, PCOLS], F32, tag="outt")
            if j % 2 == 0:
                nc.scalar.activation(outt[0:16, :], svr[0:16, cols],
                                     mybir.ActivationFunctionType.Identity,
                                     bias=b_col[0:16, :], scale=a_col[0:16, :])
                nc.scalar.activation(outt[64:112, :], svr[64:112, cols],
                                     mybir.ActivationFunctionType.Identity,
                                     bias=b_col[64:112, :],
                                     scale=a_col[64:112, :])
            else:
                nc.vector.tensor_scalar(outt[0:16, :], svr[0:16, cols],
                                        a_col[0:16, :], b_col[0:16, :],
                                        mybir.AluOpType.mult,
                                        mybir.AluOpType.add)
                nc.vector.tensor_scalar(outt[64:112, :], svr[64:112, cols],
                                        a_col[64:112, :], b_col[64:112, :],
                                        mybir.AluOpType.mult,
                                        mybir.AluOpType.add)
            nc.sync.dma_start(OUT_t[0:16, cols], outt[0:16, :])
            nc.sync.dma_start(OUT_t[16:64, cols], outt[64:112, :])

    nc.compile()
    return nc


# ---------------------------------------------------------------- host driver

def _prep_inputs(inputs, TPP):
    PCOLS = TPP * TCOLS
    NPC = PPC * PCOLS
    feats = np.asarray(inputs["feats"], np.float32)
    coords = np.asarray(inputs["coords"], np.int64)
    kbo = _make_ker_by_off(np.asarray(inputs["tp_weight"], np.float64))
    M0 = _center_matrix(kbo, np.asarray(inputs["Ws1"], np.float64),
                        np.asarray(inputs["Ws2"], np.float64),
                        np.asarray(inputs["Wv"], np.float64))
    kers64 = _build_stationaries(kbo)
    kmax = float(np.abs(kers64).max())
    sw = min(10, int(np.floor(np.log2(200.0 / max(kmax, 1e-30)))))
    wscale = float(2.0 ** sw)
    k8 = (kers64 * wscale).astype(NP8)
    res = kers64 * wscale - k8.astype(np.float64)
    kers = np.concatenate([_pack_doublerow(k8.astype(np.float64)).astype(NP8),
                           _pack_doublerow(res).astype(NP8)], axis=1)
    # center stationary in the 112-channel layout, x wscale (features carry x16)
    CK = np.zeros((32, 112), np.float32)
    CK[:, 0:16] = M0[:, 0:16]
    CK[:, 32:48] = M0[:, 16:32]
    CK[:, 64:112] = M0[:, 32:80]
    CK = (CK * wscale).astype(np.float16)
    DSC = np.full((112, 1), 1.0 / (wscale * FSCALE), np.float32)
    E = np.zeros((16, 48), np.float32)
    for w in range(16):
        for i in range(3):
            E[w, w * 3 + i] = 1.0
    BN = np.concatenate([np.asarray(inputs["bn_weight"], np.float32),
                         np.asarray(inputs["bn_bias"], np.float32)])[None, :]

    lin = (coords[:, 0] * GRID + coords[:, 1]) * GRID + coords[:, 2]
    perm = np.argsort(lin, kind="stable")
    cs = coords[perm]
    fs8 = (feats[perm] * FSCALE).astype(NP8)
    fs16 = (feats[perm] * FSCALE).astype(np.float16)

    from numpy.lib.stride_tricks import sliding_window_view
    in_maps = []
    counts = np.zeros((NCORES, PPC), np.int64)
    for c in range(NCORES):
        x0 = 16 * c
        m = (cs[:, 0] >= x0 - 2) & (cs[:, 0] < x0 + 18)
        cc, ff = cs[m], fs8[m]
        # V slots: [20 xp, 136 yp(pad), 136 zp] of 32 fp8
        Vs = np.zeros((20, 136, 136, 32), NP8)
        Vs[cc[:, 0] - x0 + 2, cc[:, 1] + 2, cc[:, 2] + 2] = ff
        Vu = Vs.view(np.uint8)
        # A4[xp, yp, s] = Vs[xp, yp, s:s+4]  -> [20,136,132,128] (dz-major, ch)
        A4 = sliding_window_view(Vu, 4, axis=2)[:, :, :SD]     # [20,136,132,32,4]
        A4 = np.ascontiguousarray(A4.transpose(0, 1, 2, 4, 3)).reshape(20, 136, SD, 128)
        # B[xp, s, yq] = concat_k A4[xp, yq+k, s]  -> [BROWS, 512] bytes
        sw_ = sliding_window_view(A4, 4, axis=1)               # [20,133,132,128,4]
        Bv = np.ascontiguousarray(
            sw_[:, :YQ].transpose(0, 2, 1, 4, 3)).reshape(BROWS, 512).view(NP8)

        mloc = (cs[:, 0] >= x0) & (cs[:, 0] < x0 + 16)
        cl = cs[mloc]
        fl = fs16[mloc]
        cfeat = np.zeros((32, NPC), np.float16)
        idx_blocks = []
        for j in range(PPC):
            pm = cl[:, 0] == x0 + j
            y, z = cl[pm, 1], cl[pm, 2]
            n = len(y)
            assert n <= PCOLS, f"plane overflow {n} > {PCOLS}"
            counts[c, j] = n
            cfeat[:, j * PCOLS: j * PCOLS + n] = fl[pm].T
            for (dx, ady, adz) in WBS:
                blk = np.full(PCOLS, PAD_IDX, np.int64)
                blk[:n] = (z + 2 + adz) * YQ + (y + 2 + ady)
                idx_blocks.append(blk)
        idx = _wrap_idx(np.concatenate(idx_blocks))
        in_maps.append({"bvol": Bv, "idx": idx, "kers": kers, "cker": CK,
                       "cfeat": cfeat, "emat": E, "bn": BN, "dsc": DSC})
    return in_maps, counts, perm, cs


def kernel(**inputs):
    global _COMPILED
    coords = np.asarray(inputs["coords"], np.int64)
    maxp = int(np.bincount(coords[:, 0], minlength=GRID).max())
    TPP = max(5, -(-maxp // TCOLS))
    if _COMPILED is None or _COMPILED[0] != TPP:
        nc = _build_program(TPP)
        _COMPILED = (TPP, nc)
    else:
        nc = _COMPILED[1]
    PCOLS = TPP * TCOLS
    in_maps, counts, perm, cs = _prep_inputs(inputs, TPP)
    res = run_bass_kernel_spmd(nc, in_maps, core_ids=list(range(NCORES)))
    pieces = []
    for c in range(NCORES):
        o = res.results[c]["out"]
        for j in range(PPC):
            n = counts[c, j]
            if n:
                pieces.append(o[:, j * PCOLS: j * PCOLS + n])
    sorted_out = np.concatenate(pieces, axis=1).T
    out = np.empty_like(sorted_out)
    out[perm] = sorted_out
    return out



# revision 10
# speedup vs baseline: 166639.5663x; 166639.5663x over previous
"""Trainium2 Bass kernel for nn_EquivariantConvolutionBlock (sparse 5^3 equivariant
conv + gate + batchnorm over 300k voxels in a 128^3 grid), SPMD over 8 NeuronCores.

v3: zero-redundancy 5-pattern gather.
- The 80 active stencil cells (0 < d^2 <= 6 within the 5^3 window) are
  partitioned into 5 fixed 16-cell patterns that may span x-planes; the host
  builds one fp8 "pattern volume" per (pattern, output plane): each 512B row
  packs exactly [16 cells x 32 ch] for one anchor voxel. Per voxel the device
  gathers 5x512B (vs v2's 8x512B window blocks) - the DMA-engine bottleneck
  drops by 3/8.
- One shared index stream per plane (all 5 patterns anchor at the voxel
  itself), so idx DMA traffic drops 8x.
- fp8 weight rounding residual bank dropped (host-measured rel err 1.6e-3
  vs the 2e-2 gate): halves PE matmul + ldweights count - PE.SEQ was 100%
  busy in v2.
- Post-processing restructured: Silu/Sigmoid applied straight from PSUM on
  ACT (no xs materialization), v-gating + s-stats on DVE in fp16 2x mode,
  v^2 stats on ACT Square+accum. Outputs stored/written fp16 (host upcasts).
- Center cell + e3nn Linear self-connection stay fp16 via a dense per-plane
  feature tile; BatchNorm batch stats AllReduce across cores.
"""
import sys

sys.path.insert(0, "/opt/trn_rl_repo")

import os
import numpy as np
from contextlib import ExitStack

import concourse.bass as bass
import concourse.bacc as bacc
import concourse.tile as tile
import concourse.mybir as mybir
from concourse.bass_utils import run_bass_kernel_spmd

F8 = mybir.dt.float8e4
F16 = mybir.dt.float16
F32 = mybir.dt.float32
I16 = mybir.dt.int16
NP8 = mybir.dt.np(F8)

N = 300000
GRID = 128
NCORES = 8
EPS = 1e-5
PPC = 16          # planes per core
NOCC = os.environ.get("NOCC", "0") == "1"
YQ = 132          # y anchor axis
SD = 132          # z anchor axis
PLANE_ROWS = SD * YQ          # 17424
NPAT = 5
TCOLS = 512
PAD_IDX = 130     # (s=0, yq=130): never a real anchor; row zeroed on host
FSCALE = 16.0     # feature scale into fp8

_COMPILED = None


def _patterns():
    """80 active cells -> 5 disjoint 16-cell patterns (may span dx planes)."""
    per_dx = {}
    for dx in range(-2, 3):
        cells = []
        for dy in range(-2, 3):
            for dz in range(-2, 3):
                d2 = dx * dx + dy * dy + dz * dz
                if 0 < d2 <= 6:
                    cells.append((dx, dy, dz))
        per_dx[dx] = cells
    m2, m1, z0, p1, p2 = (per_dx[d] for d in (-2, -1, 0, 1, 2))
    pats = [m2 + m1[:7],
            m1[7:] + z0[:2],
            z0[2:18],
            z0[18:] + p1[:14],
            p1[14:] + p2]
    assert all(len(p) == 16 for p in pats)
    assert sorted(sum(pats, [])) == sorted(sum(per_dx.values(), []))
    return pats


PATS = _patterns()


# ---------------------------------------------------------------- host math

def _soft_unit_step(t):
    out = np.zeros_like(t)
    m = t > 0
    out[m] = np.exp(-1.0 / t[m])
    return out


def _make_ker_by_off(tp_weight):
    ax = np.arange(-2, 3.0)
    lat = np.stack(np.meshgrid(ax, ax, ax, indexing="ij"), -1).reshape(-1, 3)
    d = np.linalg.norm(lat, axis=-1)
    values = np.linspace(0.0, 2.5, 5)[1:-1]
    step = 2.5 / 4
    diff = (d[..., None] - values) / step
    emb = 1.14136 * float(np.e ** 2) * _soft_unit_step(diff + 1.0) * _soft_unit_step(1.0 - diff)
    w = (emb @ tp_weight.astype(np.float64)) / 125.0
    w1, w2, w3, w4, w5, w6 = [w[:, i * 128:(i + 1) * 128].reshape(-1, 8, 16) for i in range(6)]
    unit = np.where(d[:, None] > 0, lat / np.where(d > 0, d, 1.0)[:, None], 0.0)
    y1 = np.sqrt(3.0) * unit
    A = 0.25
    B = A / np.sqrt(3.0)
    Cc = 0.25
    M_ss = A * w1
    M_vs = (B * np.einsum("xi,xuw->xuiw", y1, w2)).reshape(-1, 24, 16)
    M_sg = A * w3
    M_vg = (B * np.einsum("xi,xuw->xuiw", y1, w4)).reshape(-1, 24, 16)
    M_sv = (Cc * np.einsum("xk,xuw->xuwk", y1, w5)).reshape(-1, 8, 48)
    M_vv = (Cc * np.einsum("xuw,ik->xuiwk", w6, np.eye(3))).reshape(-1, 24, 48)
    top = np.concatenate([M_ss, M_sg, M_sv], -1)
    bot = np.concatenate([M_vs, M_vg, M_vv], -1)
    ker = np.concatenate([top, bot], 1)            # [125,32,80]
    return {tuple(int(v) for v in lat[i]): ker[i] for i in range(125)}


def _center_matrix(kbo, Ws1, Ws2, Wv):
    """Center conv cell + e3nn Linear self-connection, [32,80]."""
    inv = 1.0 / np.sqrt(8.0)
    Wsc = np.zeros((32, 80))
    Wsc[0:8, 0:16] = Ws1 * inv
    Wsc[0:8, 16:32] = Ws2 * inv
    u, w_ = np.meshgrid(np.arange(8), np.arange(16), indexing="ij")
    for i in range(3):
        Wsc[8 + u * 3 + i, 32 + w_ * 3 + i] = Wv * inv
    return kbo[(0, 0, 0)] + Wsc


def _to112(m80):
    """[.., 80] kernel cols -> 112 psum layout: 0:16 s | 32:48 gates | 64:112 v."""
    out = np.zeros(m80.shape[:-1] + (112,), m80.dtype)
    out[..., 0:16] = m80[..., 0:16]
    out[..., 32:48] = m80[..., 16:32]
    out[..., 64:112] = m80[..., 32:80]
    return out


def _build_stationaries5(kbo, wscale):
    """[128, NPAT*2*2*112] fp8-ready: slot (pat*2+qp), i in {0,1}:
    st[p, slot, i, m] = K[pattern[pat][qp*8 + (2p+i)//32]][ch=(2p+i)%32, m]*wscale.
    Matches gathered row layout byte b = cell*32 + ch (b = 256*qp + 2p + i)."""
    st = np.zeros((128, NPAT, 2, 2, 112))
    for pat in range(NPAT):
        k112 = np.stack([_to112(kbo[c]) for c in PATS[pat]], 0)  # [16,32,112]
        for i in range(2):
            u = 2 * np.arange(128) + i
            for qp in range(2):
                st[:, pat, qp, i, :] = k112[qp * 8 + u // 32, u % 32, :] * wscale
    return st.reshape(128, -1)


def _wrap_idx(flat):
    w16 = flat.reshape(-1, 16).T.astype(np.int16)
    return np.tile(w16, (8, 1))


# ---------------------------------------------------------------- device program

def _build_program(TPP):
    PCOLS = TPP * TCOLS
    NTILE = PPC * TPP
    NPC = PPC * PCOLS
    BROWS5 = NPAT * PPC * PLANE_ROWS

    NQ = 1  # >1 SWDGE queue corrupts concurrent gathers on this HW
    nc = bacc.Bacc("TRN2", target_bir_lowering=False, debug=False,
                   num_devices=NCORES, num_swdge_queues=NQ)
    B_t = nc.dram_tensor("bvol", [BROWS5, 512], F8, kind="ExternalInput").ap()
    IDX_t = nc.dram_tensor("idx", [128, PPC * PCOLS // 16], I16,
                           kind="ExternalInput").ap()
    KER_t = nc.dram_tensor("kers", [128, NPAT * 2 * 2 * 112], F8,
                           kind="ExternalInput").ap()
    CK_t = nc.dram_tensor("cker", [32, 112], F16, kind="ExternalInput").ap()
    CF_t = nc.dram_tensor("cfeat", [32, NPC], F16, kind="ExternalInput").ap()
    E_t = nc.dram_tensor("emat", [16, 48], F16, kind="ExternalInput").ap()
    BN_t = nc.dram_tensor("bn", [1, 48], F32, kind="ExternalInput").ap()
    DSC_t = nc.dram_tensor("dsc", [112, 1], F32, kind="ExternalInput").ap()
    EPS_t = nc.dram_tensor("epsu", [1, 32], F32, kind="ExternalInput").ap()
    OUT_t = nc.dram_tensor("out", [64, NPC], F16, kind="ExternalOutput").ap()

    with tile.TileContext(nc) as tc, ExitStack() as ctx:
        cpool = ctx.enter_context(tc.tile_pool(name="const", bufs=1))
        ipool = ctx.enter_context(tc.tile_pool(name="idxp", bufs=2))
        fpool = ctx.enter_context(tc.tile_pool(name="cfp", bufs=2))
        r0pool = ctx.enter_context(tc.tile_pool(name="rhs0", bufs=int(os.environ.get("KRB", "5"))))
        spool = ctx.enter_context(tc.tile_pool(name="small", bufs=3))
        qpool = ctx.enter_context(tc.tile_pool(name="sq", bufs=3))
        vpool = ctx.enter_context(tc.tile_pool(name="sv", bufs=1))
        opool = ctx.enter_context(tc.tile_pool(name="outp", bufs=int(os.environ.get("KOB", "3"))))
        stpool = ctx.enter_context(tc.tile_pool(name="stats", bufs=1))
        pp = ctx.enter_context(tc.tile_pool(name="psA", bufs=6, space="PSUM"))
        pg = ctx.enter_context(tc.tile_pool(name="psB", bufs=2, space="PSUM"))
        dpool = ctx.enter_context(tc.tile_pool(name="dram", bufs=1, space="DRAM"))

        kers = cpool.tile([128, NPAT * 2 * 2 * 112], F8)
        nc.sync.dma_start(kers[:], KER_t[:])
        cker = cpool.tile([32, 112], F16)
        nc.sync.dma_start(cker[:], CK_t[:])
        e48 = cpool.tile([48, 48], F16)
        nc.sync.dma_start(e48[32:48, :], E_t[:])
        bn = cpool.tile([1, 48], F32)
        nc.sync.dma_start(bn[:], BN_t[:])
        dsc = cpool.tile([112, 1], F32)
        nc.sync.dma_start(dsc[:], DSC_t[:])
        epsu = cpool.tile([1, 32], F32)
        nc.sync.dma_start(epsu[:], EPS_t[:])
        ones = cpool.tile([1, 1], F32)
        nc.vector.memset(ones[:], 1.0)

        stats_s = stpool.tile([16, NTILE], F32, tag="sts")
        stats_s2 = stpool.tile([16, NTILE], F32, tag="sts2")
        stats_v2 = stpool.tile([112, NTILE], F32, tag="stv2")

        # SBUF-resident gated pre-BN; rows 0:16 = s, rows 64:112 = v (engine
        # ops cannot shift partitions, so v stays on the conv output rows)
        svr = vpool.tile([112, NPC], F16, tag="svres")

        for j in range(PPC):
            ps = [pp.tile([112, TCOLS], F32, tag="convps", name=f"ps_{j}_{t}")
                  for t in range(TPP)]
            idxt = ipool.tile([128, PCOLS // 16], I16, tag="idx", name=f"ix_{j}")
            nc.sync.dma_start(idxt[:], IDX_t[:, j * PCOLS // 16:
                                             (j + 1) * PCOLS // 16])
            cft = fpool.tile([32, PCOLS], F16, tag="cf", name=f"cf_{j}")
            nc.sync.dma_start(cft[:], CF_t[:, j * PCOLS:(j + 1) * PCOLS])
            # center + self-connection matmuls open each accumulation group
            for t in range(TPP):
                nc.tensor.matmul(ps[t][:], cker[:],
                                 cft[:, t * TCOLS:(t + 1) * TCOLS],
                                 start=True, stop=False)
            for pat in range(NPAT):
                base = (pat * PPC + j) * PLANE_ROWS
                rhs = r0pool.tile([128, 4, PCOLS], F8, tag="rhs",
                                  name=f"rh_{j}_{pat}")
                nc.gpsimd.dma_gather(
                    rhs[:], B_t[base: base + PLANE_ROWS, :],
                    idxt[:], PCOLS, PCOLS, 512, transpose=True,
                    queue_num=0, single_packet=False,
                )
                rq = rhs[:].rearrange("p a n -> p (a n)").rearrange(
                    "p (q n e) -> p q e n", q=2, e=2)
                for qp in range(2):
                    k0 = (pat * 2 + qp) * 2 * 112
                    stat = kers[:, k0: k0 + 2 * 112].rearrange(
                        "p (i m) -> p i m", i=2)
                    for t in range(TPP):
                        nc.tensor.matmul(
                            ps[t][:, :],
                            stat,
                            rq[:, qp, :, t * TCOLS: (t + 1) * TCOLS],
                            start=False,
                            stop=(pat == NPAT - 1 and qp == 1),
                            perf_mode=mybir.MatmulPerfMode.DoubleRow,
                        )
            # post-processing per 512-col tile. svr stays UNSCALED (true/dsc):
            # the sigmoids see true-scale inputs via the ACT scale arg, and the
            # dsc factor cancels algebraically in the BN chain (eps -> eps/dsc^2).
            for t in range(TPP):
                ti = j * TPP + t
                cols = slice(ti * TCOLS, (ti + 1) * TCOLS)
                # one sigmoid covers silu's sigma(s) (rows 0:16) + gates (32:48)
                sig = spool.tile([48, TCOLS], F16, tag="sig")
                nc.scalar.activation(sig[:], ps[t][0:48, :],
                                     mybir.ActivationFunctionType.Sigmoid,
                                     scale=dsc[0:48, :])
                nc.vector.tensor_tensor(svr[0:16, cols], ps[t][0:16, :],
                                        sig[0:16, :], mybir.AluOpType.mult)
                gex = pg.tile([112, TCOLS], F32, tag="gexps")
                nc.tensor.matmul(gex[64:112, :], e48[32:48, :], sig[32:48, :],
                                 start=True, stop=True, tile_position=(32, 64))
                gexs = spool.tile([112, TCOLS], F16, tag="gexs")
                nc.scalar.activation(gexs[64:112, :], gex[64:112, :],
                                     mybir.ActivationFunctionType.Copy)
                nc.vector.tensor_tensor(svr[64:112, cols], ps[t][64:112, :],
                                        gexs[64:112, :], mybir.AluOpType.mult)
                # batch stats: s sum + s^2 on DVE (fp16 2x), v^2 on ACT accum
                nc.vector.tensor_reduce(stats_s[:, ti:ti + 1], svr[0:16, cols],
                                        mybir.AxisListType.X, mybir.AluOpType.add)
                sq1 = qpool.tile([16, TCOLS], F32, tag="sq1")
                nc.vector.tensor_tensor(sq1[:], svr[0:16, cols],
                                        svr[0:16, cols], mybir.AluOpType.mult)
                nc.vector.tensor_reduce(stats_s2[:, ti:ti + 1], sq1[:],
                                        mybir.AxisListType.X, mybir.AluOpType.add)
                sq2 = qpool.tile([112, TCOLS], F32, tag="sq2")
                nc.scalar.activation(sq2[64:112, :], svr[64:112, cols],
                                     mybir.ActivationFunctionType.Square,
                                     accum_out=stats_v2[64:112, ti:ti + 1])

        # ---- batch statistics: reduce partials, AllReduce, finalize scales
        red_s = stpool.tile([16, 1], F32)
        nc.vector.tensor_reduce(red_s[:], stats_s[:], mybir.AxisListType.X,
                                mybir.AluOpType.add)
        red_s2 = stpool.tile([16, 1], F32)
        nc.vector.tensor_reduce(red_s2[:], stats_s2[:], mybir.AxisListType.X,
                                mybir.AluOpType.add)
        red_v2 = stpool.tile([112, 1], F32)
        nc.vector.tensor_reduce(red_v2[64:112, :], stats_v2[64:112, :],
                                mybir.AxisListType.X, mybir.AluOpType.add)
        cc_in = dpool.tile([1, 80], F32)
        cc_out = dpool.tile([1, 80], F32)
        nc.sync.dma_start(cc_in[0:1, 0:16], red_s[:])
        nc.sync.dma_start(cc_in[0:1, 16:32], red_s2[:])
        nc.sync.dma_start(cc_in[0:1, 32:80], red_v2[64:112, :])
        if not NOCC:
            nc.gpsimd.collective_compute(
                "AllReduce", mybir.AluOpType.add,
                replica_groups=[list(range(NCORES))],
                ins=[cc_in.opt()], outs=[cc_out.opt()],
            )
        st = stpool.tile([1, 80], F32)
        nc.sync.dma_start(st[:], cc_in[:] if NOCC else cc_out[:])

        # fused s-var | v-norm rsqrt: one [1,32] Newton chain instead of two
        mu = stpool.tile([1, 16], F32)
        nc.vector.tensor_scalar_mul(mu[:], st[0:1, 0:16], 1.0 / N)
        vv = stpool.tile([1, 32], F32)
        nc.vector.tensor_scalar_mul(vv[0:1, 0:16], st[0:1, 16:32], 1.0 / N)
        mumu = stpool.tile([1, 16], F32)
        nc.vector.tensor_tensor(mumu[:], mu[:], mu[:], mybir.AluOpType.mult)
        nc.vector.tensor_tensor(vv[0:1, 0:16], vv[0:1, 0:16], mumu[:],
                                mybir.AluOpType.subtract)
        v3 = st[0:1, 32:80].rearrange("p (g d) -> p g d", d=3)
        nc.vector.tensor_reduce(vv[0:1, 16:32], v3, mybir.AxisListType.X,
                                mybir.AluOpType.add)
        nc.vector.tensor_scalar_mul(vv[0:1, 16:32], vv[0:1, 16:32],
                                    1.0 / (3.0 * N))
        t32 = stpool.tile([1, 32], F32)
        nc.vector.tensor_tensor(t32[:], vv[:], epsu[:], mybir.AluOpType.add)
        r32 = stpool.tile([1, 32], F32)
        nc.vector.reciprocal(r32[:], t32[:])
        q32 = stpool.tile([1, 32], F32)
        nc.scalar.activation(q32[:], r32[:], mybir.ActivationFunctionType.Sqrt)
        qq = stpool.tile([1, 32], F32)
        nc.vector.tensor_tensor(qq[:], q32[:], q32[:], mybir.AluOpType.mult)
        nc.vector.tensor_tensor(qq[:], qq[:], t32[:], mybir.AluOpType.mult)
        nc.vector.tensor_scalar_mul(qq[:], qq[:], -0.5)
        nc.vector.tensor_scalar_add(qq[:], qq[:], 1.5)
        ab = stpool.tile([1, 32], F32)
        nc.vector.tensor_tensor(ab[:], q32[:], qq[:], mybir.AluOpType.mult)
        nc.vector.tensor_tensor(ab[:], ab[:], bn[0:1, 0:32],
                                mybir.AluOpType.mult)
        b_s = stpool.tile([1, 16], F32)
        nc.vector.tensor_tensor(b_s[:], mu[:], ab[0:1, 0:16],
                                mybir.AluOpType.mult)
        nc.vector.tensor_tensor(b_s[:], bn[0:1, 32:48], b_s[:],
                                mybir.AluOpType.subtract)
        a_vec = stpool.tile([1, 112], F32)
        nc.vector.memset(a_vec[:], 0.0)
        nc.vector.tensor_copy(a_vec[0:1, 0:16], ab[0:1, 0:16])
        av3 = a_vec[0:1, 64:112].rearrange("p (g d) -> p g d", d=3)
        avs = ab[0:1, 16:32].rearrange("p (g d) -> p g d", d=1)
        for i in range(3):
            nc.vector.tensor_copy(av3[:, :, i:i + 1], avs[:])
        b_vec = stpool.tile([1, 112], F32)
        nc.vector.memset(b_vec[:], 0.0)
        nc.vector.tensor_copy(b_vec[0:1, 0:16], b_s[:])
        abps = pg.tile([112, 1], F32, tag="gexps")
        nc.tensor.matmul(abps[:], a_vec[:], ones[:], start=True, stop=True)
        a_col = stpool.tile([112, 1], F32)
        nc.scalar.activation(a_col[:], abps[:], mybir.ActivationFunctionType.Copy)
        abps2 = pg.tile([112, 1], F32, tag="gexps")
        nc.tensor.matmul(abps2[:], b_vec[:], ones[:], start=True, stop=True)
        b_col = stpool.tile([112, 1], F32)
        nc.scalar.activation(b_col[:], abps2[:], mybir.ActivationFunctionType.Copy)

        # ---- apply normalization: out = sv * a + b (from SBUF)
        # alternate planes between ACT and DVE so the post-barrier tail halves
        for j in range(PPC):
            cols = slice(j * PCOLS, (j + 1) * PCOLS)
            outt = opool.tile([112, PCOLS], F16, tag="outt")
            if j % 2 == 0:
                nc.scalar.activation(outt[0:16, :], svr[0:16, cols],
                                     mybir.ActivationFunctionType.Identity,
                                     bias=b_col[0:16, :], scale=a_col[0:16, :])
                nc.scalar.activation(outt[64:112, :], svr[64:112, cols],
                                     mybir.ActivationFunctionType.Identity,
                                     bias=b_col[64:112, :],
                                     scale=a_col[64:112, :])
            else:
                nc.vector.tensor_scalar(outt[0:16, :], svr[0:16, cols],
                                        a_col[0:16, :], b_col[0:16, :],
                                        mybir.AluOpType.mult,
                                        mybir.AluOpType.add)
                nc.vector.tensor_scalar(outt[64:112, :], svr[64:112, cols],
                                        a_col[64:112, :], b_col[64:112, :],
                                        mybir.AluOpType.mult,
                                        mybir.AluOpType.add)
            nc.sync.dma_start(OUT_t[0:16, cols], outt[0:16, :])
            nc.sync.dma_start(OUT_t[16:64, cols], outt[64:112, :])

    nc.compile()
    return nc


# ---------------------------------------------------------------- host driver

def _prep_inputs(inputs, TPP):
    PCOLS = TPP * TCOLS
    NPC = PPC * PCOLS
    feats = np.asarray(inputs["feats"], np.float32)
    coords = np.asarray(inputs["coords"], np.int64)
    kbo = _make_ker_by_off(np.asarray(inputs["tp_weight"], np.float64))
    M0 = _center_matrix(kbo, np.asarray(inputs["Ws1"], np.float64),
                        np.asarray(inputs["Ws2"], np.float64),
                        np.asarray(inputs["Wv"], np.float64))
    kmax = max(float(np.abs(kbo[c]).max()) for p in PATS for c in p)
    # cap wscale*FSCALE at 512 so the unscaled fp16 svr (= true * wscale *
    # FSCALE) stays far from fp16 max; fp8 weight precision is scale-free.
    sw = min(5, int(np.floor(np.log2(200.0 / max(kmax, 1e-30)))))
    wscale = float(2.0 ** sw)
    kers = _build_stationaries5(kbo, wscale).astype(NP8)
    # center stationary in the 112-channel layout, x wscale (features carry x16)
    CK = (_to112(M0) * wscale).astype(np.float16)
    DSC = np.full((112, 1), 1.0 / (wscale * FSCALE), np.float32)
    EPSU = np.full((1, 32), EPS * (wscale * FSCALE) ** 2, np.float32)
    E = np.zeros((16, 48), np.float16)
    for w in range(16):
        for i in range(3):
            E[w, w * 3 + i] = 1.0
    BN = np.concatenate([np.asarray(inputs["bn_weight"], np.float32),
                         np.asarray(inputs["bn_bias"], np.float32)])[None, :]

    lin = (coords[:, 0] * GRID + coords[:, 1]) * GRID + coords[:, 2]
    perm = np.argsort(lin, kind="stable")
    cs = coords[perm]
    fs8 = (feats[perm] * FSCALE).astype(NP8)
    fs16 = (feats[perm] * FSCALE).astype(np.float16)

    in_maps = []
    counts = np.zeros((NCORES, PPC), np.int64)
    for c in range(NCORES):
        x0 = 16 * c
        m = (cs[:, 0] >= x0 - 2) & (cs[:, 0] < x0 + 18)
        cc, ff = cs[m], fs8[m]
        # dense slab with +-2 halo: [20 xp, 136 y(pad), 136 z(pad), 32ch]
        Vs = np.zeros((20, 136, 136, 32), NP8)
        Vs[cc[:, 0] - x0 + 2, cc[:, 1] + 2, cc[:, 2] + 2] = ff
        # pattern volumes: row (pat, j, s, yq) = 16 cells x 32 ch fp8
        V5 = np.zeros((NPAT, PPC, SD, YQ, 16, 32), NP8)
        for pat in range(NPAT):
            for ci, (dx, dy, dz) in enumerate(PATS[pat]):
                V5[pat, :, 2:130, 2:130, ci, :] = Vs[
                    2 + dx:18 + dx, 2 + dy:130 + dy, 2 + dz:130 + dz, :
                ].transpose(0, 2, 1, 3)
        Bv = V5.reshape(NPAT * PPC * PLANE_ROWS, 512)

        mloc = (cs[:, 0] >= x0) & (cs[:, 0] < x0 + 16)
        cl = cs[mloc]
        fl = fs16[mloc]
        cfeat = np.zeros((32, NPC), np.float16)
        idx_blocks = []
        for j in range(PPC):
            pm = cl[:, 0] == x0 + j
            y, z = cl[pm, 1], cl[pm, 2]
            n = len(y)
            assert n <= PCOLS, f"plane overflow {n} > {PCOLS}"
            counts[c, j] = n
            cfeat[:, j * PCOLS: j * PCOLS + n] = fl[pm].T
            blk = np.full(PCOLS, PAD_IDX, np.int64)
            blk[:n] = (z + 2) * YQ + (y + 2)
            idx_blocks.append(blk)
        idx = _wrap_idx(np.concatenate(idx_blocks))
        in_maps.append({"bvol": Bv, "idx": idx, "kers": kers, "cker": CK,
                       "cfeat": cfeat, "emat": E, "bn": BN, "dsc": DSC,
                       "epsu": EPSU})
    return in_maps, counts, perm, cs


def kernel(**inputs):
    global _COMPILED
    coords = np.asarray(inputs["coords"], np.int64)
    maxp = int(np.bincount(coords[:, 0], minlength=GRID).max())
    TPP = max(5, -(-maxp // TCOLS))
    if _COMPILED is None or _COMPILED[0] != TPP:
        nc = _build_program(TPP)
        _COMPILED = (TPP, nc)
    else:
        nc = _COMPILED[1]
    PCOLS = TPP * TCOLS
    in_maps, counts, perm, cs = _prep_inputs(inputs, TPP)
    res = run_bass_kernel_spmd(nc, in_maps, core_ids=list(range(NCORES)))
    pieces = []
    for c in range(NCORES):
        o = res.results[c]["out"]
        for j in range(PPC):
            n = counts[c, j]
            if n:
                pieces.append(o[:, j * PCOLS: j * PCOLS + n])
    sorted_out = np.concatenate(pieces, axis=1).T.astype(np.float32)
    out = np.empty_like(sorted_out)
    out[perm] = sorted_out
    return out
